# revision 3
# baseline (speedup 1.0000x reference)
"""BatchAll triplet loss (multi-module variant) on 8 Trainium2 NeuronCores.

Math: labels = [0..191, 0..191] -- each anchor i has exactly ONE valid positive
j = (i+192) % 384, so the (i,j,k) cubic triplet tensor collapses to (i,k):

    loss_terms[i,k] = relu(d(i, p(i)) - d(i,k) + margin) * w[i,k] * valid[i,k]
    out = sum(loss_terms) / (count(loss_terms > EPS) + EPS)

d(i,k) = sqrt(max(2 + delta - 2*G[i,k]*rn_i*rn_k, 0)) with raw fp8 Gram G and
rn = 1/||e||; the min-0 clamp (in negated form) guards the masked diagonal.

v2 design (vs the ER+ET baseline):
- Only ONE input tensor of embeddings (ET, the transposed [128,1536] fp8
  layout).  Norms come from ET: elementwise squares (DVE chunks 0,2 / ACT
  chunks 1,3 -> bf16) PE-reduced with a ones column into a [1,384] PSUM row.
  This drops 196KB of input DMA and the 3 rn transposes of the baseline.
- ET ships as two [128,768] column-halves on the sync ring so squaring of
  half 0 overlaps the transfer of half 1; pm follows on the same ring.
- rn' = sqrt(2)/||e|| via sqrt(0.5*ns) (scale folded into the ACT sqrt) +
  DVE reciprocal of the [1,384] row.
- RB2[p,f] = rn'_a[p]*rn'_k[f] via a single-contraction outer-product
  matmul (rn_row[0:64] x rn_row blocks) -- replaces the baseline's selector
  matmul + rnAsb copy + x1 ACT scale + 4 broadcast panels.
- t2 = G (.) RB2 = 2*ghat; d2n = (t2 - (2+delta)) min 0 = -d^2;
  dms = sqrt(-1 * d2n) on ACT (scale=-1 folds the negation).
- dpos^2 = (2+delta) - t2_pos: tp_ps prefilled with 2+delta (rank-1 matmul),
  selector matmul accumulates -t2_pos (sign folded into the diag-extract STT).
- lwpre = (dms - dposm) * pmneg on DVE; sum via DVE max-accum; count via ACT
  sign-accum (count = (signsum + cells)/2 on host).
- Single DMA ring (sync) for everything; ACT keeps only its table load(s)
  before the squares.
"""

import os
import sys

for _p in ("/opt/trn_rl_repo", "/root/.axon_site/_ro/trn_rl_repo"):
    if _p not in sys.path:
        sys.path.append(_p)

if "jax" not in sys.modules and os.environ.get("JAX_PLATFORMS") in ("cpu",):
    del os.environ["JAX_PLATFORMS"]

import ml_dtypes
import numpy as np

import concourse.bass as bass
import concourse.tile as tile
from concourse import mybir
from concourse.bacc import Bacc
from concourse.bass_utils import run_bass_kernel_spmd

F32 = mybir.dt.float32
BF16 = mybir.dt.bfloat16
F8 = mybir.dt.float8e4
ALU = mybir.AluOpType
ACT = mybir.ActivationFunctionType

B = 192
N = 2 * B
D = 512
NCORES = 8
S = N // NCORES          # 48 anchors per core
MARGIN = 0.1
EPS = 1e-8
DELTA = 1e-5
CELLS = 128 * 192 * NCORES
N_WARMUP = 4


def build_nc() -> bass.Bass:
    nc = Bacc()

    et = nc.dram_tensor("et", [128, 1536], F8, kind="ExternalInput")
    pmw = nc.dram_tensor("pmw", [128, 192], BF16, kind="ExternalInput")
    out = nc.dram_tensor("out", [1, 2], F32, kind="ExternalOutput")

    with tile.TileContext(nc) as tc:
        with (
            tc.tile_pool(name="sb", bufs=1) as sb,
            tc.tile_pool(name="ps", bufs=1, space="PSUM") as ps,
        ):
            ET = sb.tile([128, 1536], F8, tag="ET")
            pm = sb.tile([128, 192], BF16, tag="pm")

            # ---- DMAs, all on the sync ring, in need-order.  ET ships as two
            #      column-halves so squaring h0 overlaps the h1 transfer. ----
            nc.sync.dma_start(out=ET[:, 0:768], in_=et[:, 0:768])
            nc.sync.dma_start(out=ET[:, 768:1536], in_=et[:, 768:1536])
            nc.sync.dma_start(out=pm, in_=pmw[:, :])

            # ---- identity via iota on the otherwise-idle gpsimd ----
            icol = sb.tile([128, 128], F32, tag="icol")
            nc.gpsimd.iota(icol, [[1, 128]], channel_multiplier=0,
                           allow_small_or_imprecise_dtypes=True)
            iprt = sb.tile([128, 1], F32, tag="iprt")
            nc.gpsimd.iota(iprt, [[0, 1]], channel_multiplier=1,
                           allow_small_or_imprecise_dtypes=True)
            identB = sb.tile([128, 128], BF16, tag="ident")
            nc.gpsimd.tensor_scalar(identB, icol, iprt, None, op0=ALU.is_equal)

            # ---- DVE constants ----
            wtile = sb.tile([128, 256], F8, tag="wtile")
            nc.vector.memset(wtile, 1.0)
            onesc = sb.tile([128, 1], BF16, tag="onesc")
            nc.vector.memset(onesc, 1.0)
            beps = sb.tile([128, 1], F32, tag="beps")
            nc.vector.memset(beps, -EPS)
            ones1 = sb.tile([1, 128], BF16, tag="ones1")
            nc.vector.memset(ones1, 1.0)
            b2c = sb.tile([1, 1], BF16, tag="b2c")
            nc.vector.memset(b2c, 2.0 + DELTA)
            tdsrc = sb.tile([1, 1], F32, tag="tdsrc")
            nc.vector.memset(tdsrc, 1.0)

            # ---- dummy sqrt FIRST on ACT: makes the table-load pass pick
            #      set 3 (sqrt_and_others), which also serves Square and
            #      Sign -- ONE 1.28us table load instead of two. ----
            tdum = sb.tile([1, 1], F32, tag="tdum")
            nc.scalar.sqrt(tdum, tdsrc)

            # ---- PE warm-up bridging the DMA phase ----
            wps = ps.tile([128, 256], F32, tag="wps")
            for _ in range(N_WARMUP):
                nc.tensor.matmul(wps, wtile[:, 0:128], wtile,
                                 start=True, stop=True)

            # ---- squares of ET chunks: DVE c0,c2 / ACT c1,c3 -> bf16 ----
            sq = sb.tile([128, 1536], BF16, tag="sq")
            with nc.allow_low_precision("bf16 squares; averaged over 512 dims"):
                nc.vector.scalar_tensor_tensor(
                    sq[:, 0:384], ET[:, 0:384], 1.0, ET[:, 0:384],
                    op0=ALU.mult, op1=ALU.mult)
                nc.scalar.activation(sq[:, 384:768], ET[:, 384:768], ACT.Square)
                nc.vector.scalar_tensor_tensor(
                    sq[:, 768:1152], ET[:, 768:1152], 1.0, ET[:, 768:1152],
                    op0=ALU.mult, op1=ALU.mult)
                nc.scalar.activation(sq[:, 1152:1536], ET[:, 1152:1536],
                                     ACT.Square)

            # ---- Gram in [128,192] layout: 2 blocks x 4 chunks, fp8.
            #      Emitted interleaved with the ns reduction matmuls so the
            #      in-order PE stream never blocks on a not-yet-ready sq
            #      chunk while Gram work is available. ----
            g_ps = ps.tile([128, 192], F32, tag="G")
            ns_ps = ps.tile([1, 384], F32, tag="ns")

            def gram(c):
                lhsT = ET[:, 384 * c:384 * c + 64]
                nc.tensor.matmul(g_ps[0:64, :], lhsT,
                                 ET[:, 384 * c:384 * c + 192],
                                 start=(c == 0), stop=(c == 3),
                                 skip_group_check=True)
                nc.tensor.matmul(g_ps[64:128, :], lhsT,
                                 ET[:, 384 * c + 192:384 * c + 384],
                                 start=(c == 0), stop=(c == 3),
                                 skip_group_check=True)

            def nsred(c):
                nc.tensor.matmul(ns_ps, onesc, sq[:, 384 * c:384 * (c + 1)],
                                 start=(c == 0), stop=(c == 3),
                                 skip_group_check=True)

            gram(0)
            gram(1)
            nsred(0)
            nsred(1)
            gram(2)
            gram(3)
            nsred(2)
            nsred(3)

            # ---- prefill tp_ps = 2+delta (rank-1, off the critical path) ----
            tp_ps = ps.tile([128, 1], F32, tag="tp")
            nc.tensor.matmul(tp_ps, ones1, b2c, start=True, stop=False,
                             skip_group_check=True)

            # ---- G -> SBUF (frees the DVE TT below to read one PSUM src) ----
            gsb = sb.tile([128, 192], F32, tag="gsb")
            nc.vector.tensor_copy(gsb, g_ps)

            # ---- selector tile from ident: sel2[c,p]=1 iff p%64==c ----
            sel2 = sb.tile([48, 128], BF16, tag="sel2")
            nc.vector.memset(sel2, 0.0)
            nc.vector.tensor_copy(sel2[:, 0:48], identB[0:48, 0:48])
            nc.vector.tensor_copy(sel2[:, 64:112], identB[0:48, 0:48])

            # ---- rn' = sqrt(2)/||e||: sqrt(0.5*ns) on ACT, recip on DVE ----
            nrm_row = sb.tile([1, 384], F32, tag="nrm_row")
            nc.scalar.activation(nrm_row, ns_ps, ACT.Sqrt, bias=0.0, scale=0.5)
            rn_row = sb.tile([1, 384], BF16, tag="rn_row")
            with nc.allow_low_precision("bf16 rn; clamp-guarded downstream"):
                nc.vector.reciprocal(rn_row, nrm_row)

            # ---- RB2[p,f] = rn'_a[p] * rn'_k[f] via outer-product matmuls.
            #      Partition block 0 anchors = local emb 0:64, block 1 the
            #      same (pad rows land on pos norms, masked by pm=0). ----
            rb2_ps = ps.tile([128, 192], F32, tag="RB2")
            nc.tensor.matmul(rb2_ps[0:64, :], rn_row[0:1, 0:64],
                             rn_row[0:1, 0:192], start=True, stop=True,
                             skip_group_check=True)
            nc.tensor.matmul(rb2_ps[64:128, :], rn_row[0:1, 0:64],
                             rn_row[0:1, 192:384], start=True, stop=True,
                             skip_group_check=True)

            # ---- t2 = G * RB2 = 2*ghat ----
            t2s = sb.tile([128, 192], F32, tag="t2s")
            nc.vector.tensor_mul(t2s, gsb, rb2_ps)

            # ---- positive-pair diag -> -t2_pos (sign folded via scalar) ----
            tpj = sb.tile([48, 48], F32, tag="tpj")
            t2pos = sb.tile([48, 1], BF16, tag="t2pos")
            with nc.allow_low_precision("bf16 dpos path; |err| ~3e-3 abs"):
                nc.vector.scalar_tensor_tensor(
                    tpj, t2s[0:48, 48:96], -1.0, identB[0:48, 0:48],
                    op0=ALU.mult, op1=ALU.mult, accum_out=t2pos)

            # ---- d2n = (t2 - (2+delta)) min 0  =  -d^2 (clamped) ----
            d2n = sb.tile([128, 192], F32, tag="d2n")
            nc.vector.tensor_scalar(
                d2n, t2s, 2.0 + DELTA, 0.0, op0=ALU.subtract, op1=ALU.min)

            # ---- dpos^2 = 2+delta - t2_pos via accumulating selector mm ----
            nc.tensor.matmul(tp_ps, sel2, t2pos, start=False, stop=True,
                             skip_group_check=True)
            dpos = sb.tile([128, 1], F32, tag="dpos")
            nc.scalar.sqrt(dpos, tp_ps)

            # ---- d grid: sqrt(-1 * d2n) on ACT ----
            dms = sb.tile([128, 192], F32, tag="dms")
            nc.scalar.activation(dms, d2n, ACT.Sqrt, bias=0.0, scale=-1.0)
            dposm = sb.tile([128, 1], F32, tag="dposm")
            nc.vector.tensor_scalar_add(dposm, dpos, MARGIN)

            # ---- weighted terms; sum on DVE, sign-count on ACT ----
            lwpre = sb.tile([128, 192], F32, tag="lwpre")
            nc.vector.scalar_tensor_tensor(
                lwpre, dms, dposm, pm, op0=ALU.subtract, op1=ALU.mult)
            stacked = sb.tile([128, 2], BF16, tag="stacked")
            lwj = sb.tile([128, 192], F32, tag="lwj")
            sgj = sb.tile([128, 192], F32, tag="sgj")
            with nc.allow_low_precision(
                    "bf16 partials: sign-sums are integers < 256 (exact); "
                    "lw-sums carry ~0.4% rounding, ~0.05% on the total"):
                nc.vector.tensor_scalar(
                    lwj, lwpre, 0.0, 0.0, op0=ALU.max, op1=ALU.add,
                    accum_out=stacked[:, 0:1])
                nc.scalar.activation(sgj, lwpre, ACT.Sign, bias=beps,
                                     scale=1.0, accum_out=stacked[:, 1:2])

            # ---- cross-partition reduce + writeback ----
            outp = ps.tile([1, 2], F32, tag="outp")
            nc.tensor.matmul(outp, onesc, stacked, start=True, stop=True,
                             skip_group_check=True)
            outs = sb.tile([1, 2], F32, tag="outs")
            nc.vector.tensor_copy(outs, outp)
            nc.sync.dma_start(out=out[:, :], in_=outs)

    nc.finalize()
    return nc


_NC_CACHE: dict = {}


def _get_nc() -> bass.Bass:
    if "nc" not in _NC_CACHE:
        _NC_CACHE["nc"] = build_nc()
    return _NC_CACHE["nc"]


def make_in_maps(output1, output2, weight):
    o1 = np.asarray(output1, dtype=np.float32)
    o2 = np.asarray(output2, dtype=np.float32)
    w = np.asarray(weight, dtype=np.float32)

    emb = np.concatenate([o1, o2], axis=0)
    w2 = np.tile(w, (2, 2))
    f8 = ml_dtypes.float8_e4m3
    a48 = np.arange(S)

    in_maps = []
    for c in range(NCORES):
        anchors = np.arange(c * S, c * S + S)
        pos = (anchors + B) % N
        used = np.zeros(N, dtype=bool)
        used[anchors] = True
        used[pos] = True
        loc = np.concatenate([anchors, pos, np.nonzero(~used)[0]])

        emb_loc = np.ascontiguousarray(emb[loc])
        embt = emb_loc.T
        ET = np.concatenate([embt[128 * k:128 * (k + 1), :] for k in range(4)],
                            axis=1).astype(f8)

        pmn = np.zeros((128, 192), dtype=np.float32)
        pmn[0:48, :] = -w2[anchors[:, None], loc[None, 0:192]]
        pmn[64:112, :] = -w2[anchors[:, None], loc[None, 192:384]]
        pmn[a48, a48] = 0.0          # k == i
        pmn[a48, S + a48] = 0.0      # k == p(i)

        in_maps.append({
            "et": ET,
            "pmw": pmn.astype(ml_dtypes.bfloat16),
        })
    return in_maps


def reduce_outputs(results):
    parts = np.stack([np.asarray(r["out"][0], dtype=np.float64)
                      for r in results])
    total = parts.sum(axis=0)
    count = (total[1] + CELLS) / 2.0
    return np.asarray(
        np.float32(total[0]) / (np.float32(count) + np.float32(EPS)),
        dtype=np.float32)


def kernel(output1, output2, weight):
    in_maps = make_in_maps(output1, output2, weight)
    res = run_bass_kernel_spmd(_get_nc(), in_maps, core_ids=list(range(NCORES)))
    return reduce_outputs(res.results)


# revision 4
# speedup vs baseline: 1.1426x; 1.1426x over previous
"""BatchAll triplet loss (multi-module variant) on 8 Trainium2 NeuronCores.

Math: labels = [0..191, 0..191] -- each anchor i has exactly ONE valid positive
j = (i+192) % 384, so the (i,j,k) cubic triplet tensor collapses to (i,k):

    loss_terms[i,k] = relu(d(i, p(i)) - d(i,k) + margin) * w[i,k] * valid[i,k]
    out = sum(loss_terms) / (count(loss_terms > EPS) + EPS)

d(i,k) = sqrt(max(2 + delta - 2*G[i,k]*rn_i*rn_k, 0)) with raw fp8 Gram G and
rn = 1/||e||; the min-0 clamp (in negated form) guards the masked diagonal.

v2 design (vs the ER+ET baseline):
- Only ONE embeddings input (ET, transposed [128,1536] fp8) as a SINGLE DMA
  (128 packets -- the per-packet pitch is row-count-bound, so column splits
  only add packets).  Norms from ET: fp8 squares (DVE c0,c2 / ACT c1,c3)
  PE-reduced with an fp8 ones column into a [1,384] PSUM row.
- sels [48,128] selector/identity constant ships as an input (the gpsimd
  iota+is_equal path took 2.2us and stalled the DVE via the scheduler).
  gpsimd keeps one dummy memset so the Pool engine stays in the NEFF (the
  NRT postamble splits the 253-semaphore reset sweep across engines).
- One ACT table load: a dummy sqrt is the first ACT op, so the table pass
  picks set 3 (sqrt_and_others) which also serves Square and Sign.
- nrm' = sqrt(0.5*ns) row (bf16), outer product O = nrm'_a (x) nrm'_k via 2
  rank-1 matmuls, R = 1/O via reciprocal_approx_fast on the [128,192] grid
  (the [1,384] row InstReciprocal measured 2541ns -- single lane).
- t2 = G (.) R = 2*ghat; d2n = (t2 - (2+delta)) min 0; dms = sqrt(-d2n).
- dpos path off the critical path: ngpos = -diag(G_pos) extracted in DVE
  slack right after the Gram; t2pos = R_pos*ngpos in one STT after R;
  selector matmul accumulates onto a 2+delta prefill; ACT sqrt BEFORE dms.
- lwpre = (dms - dposm)*pmneg; sum via DVE max-accum; count via ACT
  sign-accum (count = (signsum + cells)/2 on host).
"""

import os
import sys

for _p in ("/opt/trn_rl_repo", "/root/.axon_site/_ro/trn_rl_repo"):
    if _p not in sys.path:
        sys.path.append(_p)

if "jax" not in sys.modules and os.environ.get("JAX_PLATFORMS") in ("cpu",):
    del os.environ["JAX_PLATFORMS"]

import ml_dtypes
import numpy as np

import concourse.bass as bass
import concourse.tile as tile
from concourse import mybir
from concourse.bacc import Bacc
from concourse.bass_utils import run_bass_kernel_spmd

F32 = mybir.dt.float32
BF16 = mybir.dt.bfloat16
F8 = mybir.dt.float8e4
ALU = mybir.AluOpType
ACT = mybir.ActivationFunctionType

B = 192
N = 2 * B
D = 512
NCORES = 8
S = N // NCORES          # 48 anchors per core
MARGIN = 0.1
EPS = 1e-8
DELTA = 1e-5
CELLS = 128 * 192 * NCORES
N_WARMUP = 4
FUSED_DMS = False        # sqrt(bias + scale*t2) without the min-0 clamp


def build_nc() -> bass.Bass:
    nc = Bacc()

    et = nc.dram_tensor("et", [128, 1536], F8, kind="ExternalInput")
    pmw = nc.dram_tensor("pmw", [128, 192], BF16, kind="ExternalInput")
    selw = nc.dram_tensor("selw", [48, 128], BF16, kind="ExternalInput")
    out = nc.dram_tensor("out", [1, 2], F32, kind="ExternalOutput")

    with tile.TileContext(nc) as tc:
        with (
            tc.tile_pool(name="sb", bufs=1) as sb,
            tc.tile_pool(name="ps", bufs=1, space="PSUM") as ps,
        ):
            ET = sb.tile([128, 1536], F8, tag="ET")
            pm = sb.tile([128, 192], BF16, tag="pm")
            sels = sb.tile([48, 128], BF16, tag="sels")

            # ---- DMAs, all on the sync ring, in need-order ----
            nc.sync.dma_start(out=ET, in_=et[:, :])
            nc.sync.dma_start(out=pm, in_=pmw[:, :])
            nc.sync.dma_start(out=sels, in_=selw[:, :])

            # ---- keep the Pool engine present in the NEFF ----
            pooldum = sb.tile([1, 1], F32, tag="pooldum")
            nc.gpsimd.memset(pooldum, 0.0)

            # ---- DVE constants ----
            wtile = sb.tile([128, 256], F8, tag="wtile")
            nc.vector.memset(wtile, 1.0)
            onesf8 = sb.tile([128, 1], F8, tag="onesf8")
            nc.vector.memset(onesf8, 1.0)
            onescb = sb.tile([128, 1], BF16, tag="onescb")
            nc.vector.memset(onescb, 1.0)
            beps = sb.tile([128, 1], F32, tag="beps")
            nc.vector.memset(beps, -EPS)
            ones1 = sb.tile([1, 128], BF16, tag="ones1")
            nc.vector.memset(ones1, 1.0)
            b2c = sb.tile([1, 1], BF16, tag="b2c")
            nc.vector.memset(b2c, 2.0 + DELTA)
            tdsrc = sb.tile([1, 1], F32, tag="tdsrc")
            nc.vector.memset(tdsrc, 1.0)

            # ---- dummy sqrt FIRST on ACT -> single set-3 table load ----
            tdum = sb.tile([1, 1], F32, tag="tdum")
            nc.scalar.sqrt(tdum, tdsrc)

            # ---- PE warm-up bridging the DMA phase ----
            wps = ps.tile([128, 256], F32, tag="wps")
            for _ in range(N_WARMUP):
                nc.tensor.matmul(wps, wtile[:, 0:128], wtile,
                                 start=True, stop=True)

            # ---- squares of ET chunks -> fp8 (averaged over 512 dims) ----
            sq = sb.tile([128, 1536], F8, tag="sq")
            with nc.allow_low_precision("fp8 squares; averaged over 512 dims"):
                nc.vector.scalar_tensor_tensor(
                    sq[:, 0:384], ET[:, 0:384], 1.0, ET[:, 0:384],
                    op0=ALU.mult, op1=ALU.mult)
                nc.scalar.activation(sq[:, 384:768], ET[:, 384:768], ACT.Square)
                nc.vector.scalar_tensor_tensor(
                    sq[:, 768:1152], ET[:, 768:1152], 1.0, ET[:, 768:1152],
                    op0=ALU.mult, op1=ALU.mult)
                nc.scalar.activation(sq[:, 1152:1536], ET[:, 1152:1536],
                                     ACT.Square)

            # ---- PE: full Gram first (gated only on ET), then ns ----
            g_ps = ps.tile([128, 192], F32, tag="G")
            for c in range(4):
                lhsT = ET[:, 384 * c:384 * c + 64]
                nc.tensor.matmul(g_ps[0:64, :], lhsT,
                                 ET[:, 384 * c:384 * c + 192],
                                 start=(c == 0), stop=(c == 3),
                                 skip_group_check=True)
                nc.tensor.matmul(g_ps[64:128, :], lhsT,
                                 ET[:, 384 * c + 192:384 * c + 384],
                                 start=(c == 0), stop=(c == 3),
                                 skip_group_check=True)

            ns_ps = ps.tile([1, 384], F32, tag="ns")
            for c in range(4):
                nc.tensor.matmul(ns_ps, onesf8, sq[:, 384 * c:384 * (c + 1)],
                                 start=(c == 0), stop=(c == 3),
                                 skip_group_check=True)

            # ---- prefill tp_ps = 2+delta (rank-1, off the critical path) ----
            tp_ps = ps.tile([128, 1], F32, tag="tp")
            nc.tensor.matmul(tp_ps, ones1, b2c, start=True, stop=False,
                             skip_group_check=True)

            # ---- ngpos = -diag(G[0:48, 48:96]) in DVE slack ----
            gj = sb.tile([48, 48], F32, tag="gj")
            ngpos = sb.tile([48, 1], F32, tag="ngpos")
            nc.vector.scalar_tensor_tensor(
                gj, g_ps[0:48, 48:96], -1.0, sels[:, 0:48],
                op0=ALU.mult, op1=ALU.mult, accum_out=ngpos)

            # ---- nrm' = sqrt(0.5*ns) row in bf16 ----
            nrow = sb.tile([1, 384], BF16, tag="nrow")
            with nc.allow_low_precision("bf16 norms; clamp-guarded"):
                nc.scalar.activation(nrow, ns_ps, ACT.Sqrt, bias=0.0,
                                     scale=0.5)

            # ---- O = nrm'_a (x) nrm'_k via 2 outer-product matmuls ----
            o_ps = ps.tile([128, 192], F32, tag="O")
            nc.tensor.matmul(o_ps[0:64, :], nrow[0:1, 0:64],
                             nrow[0:1, 0:192], start=True, stop=True,
                             skip_group_check=True)
            nc.tensor.matmul(o_ps[64:128, :], nrow[0:1, 0:64],
                             nrow[0:1, 192:384], start=True, stop=True,
                             skip_group_check=True)

            # ---- R = 1/O = 2*rn_a*rn_k ----
            rgrid = sb.tile([128, 192], F32, tag="rgrid")
            nc.vector.reciprocal_approx_fast(rgrid, o_ps)

            # ---- t2pos = R_pos * ngpos (one STT), off critical path ----
            rj = sb.tile([48, 48], F32, tag="rj")
            t2pos = sb.tile([48, 1], BF16, tag="t2pos")
            with nc.allow_low_precision("bf16 dpos path; |err| ~3e-3 abs"):
                nc.vector.scalar_tensor_tensor(
                    rj, rgrid[0:48, 48:96], ngpos, sels[:, 0:48],
                    op0=ALU.mult, op1=ALU.mult, accum_out=t2pos)

            # ---- t2 = G * R = 2*ghat ----
            t2s = sb.tile([128, 192], F32, tag="t2s")
            nc.vector.tensor_mul(t2s, g_ps, rgrid)

            # ---- dpos^2 = 2+delta - t2_pos via accumulating selector mm ----
            nc.tensor.matmul(tp_ps, sels, t2pos, start=False, stop=True,
                             skip_group_check=True)
            dpos = sb.tile([128, 1], F32, tag="dpos")
            nc.scalar.sqrt(dpos, tp_ps)

            if FUSED_DMS:
                dms = sb.tile([128, 192], F32, tag="dms")
                nc.scalar.activation(dms, t2s, ACT.Sqrt, bias=2.0 + DELTA,
                                     scale=-1.0)
            else:
                d2n = sb.tile([128, 192], F32, tag="d2n")
                nc.vector.tensor_scalar(
                    d2n, t2s, 2.0 + DELTA, 0.0, op0=ALU.subtract, op1=ALU.min)
                dms = sb.tile([128, 192], F32, tag="dms")
                nc.scalar.activation(dms, d2n, ACT.Sqrt, bias=0.0, scale=-1.0)

            dposm = sb.tile([128, 1], F32, tag="dposm")
            nc.vector.tensor_scalar_add(dposm, dpos, MARGIN)

            # ---- weighted terms; sum on DVE, sign-count on ACT ----
            lwpre = sb.tile([128, 192], F32, tag="lwpre")
            nc.vector.scalar_tensor_tensor(
                lwpre, dms, dposm, pm, op0=ALU.subtract, op1=ALU.mult)
            stacked = sb.tile([128, 2], BF16, tag="stacked")
            lwj = sb.tile([128, 192], F32, tag="lwj")
            sgj = sb.tile([128, 192], F32, tag="sgj")
            with nc.allow_low_precision(
                    "bf16 partials: sign-sums are integers < 256 (exact); "
                    "lw-sums carry ~0.4% rounding, ~0.05% on the total"):
                nc.vector.tensor_scalar(
                    lwj, lwpre, 0.0, 0.0, op0=ALU.max, op1=ALU.add,
                    accum_out=stacked[:, 0:1])
                nc.scalar.activation(sgj, lwpre, ACT.Sign, bias=beps,
                                     scale=1.0, accum_out=stacked[:, 1:2])

            # ---- cross-partition reduce + writeback ----
            outp = ps.tile([1, 2], F32, tag="outp")
            nc.tensor.matmul(outp, onescb, stacked, start=True, stop=True,
                             skip_group_check=True)
            outs = sb.tile([1, 2], F32, tag="outs")
            nc.vector.tensor_copy(outs, outp)
            nc.sync.dma_start(out=out[:, :], in_=outs)

    nc.finalize()
    return nc


_NC_CACHE: dict = {}


def _get_nc() -> bass.Bass:
    if "nc" not in _NC_CACHE:
        _NC_CACHE["nc"] = build_nc()
    return _NC_CACHE["nc"]


def _sels_const() -> np.ndarray:
    s = np.zeros((48, 128), dtype=np.float32)
    i = np.arange(48)
    s[i, i] = 1.0
    s[i, 64 + i] = 1.0
    return s.astype(ml_dtypes.bfloat16)


def make_in_maps(output1, output2, weight):
    o1 = np.asarray(output1, dtype=np.float32)
    o2 = np.asarray(output2, dtype=np.float32)
    w = np.asarray(weight, dtype=np.float32)

    emb = np.concatenate([o1, o2], axis=0)
    w2 = np.tile(w, (2, 2))
    f8 = ml_dtypes.float8_e4m3
    a48 = np.arange(S)
    sels = _sels_const()

    in_maps = []
    for c in range(NCORES):
        anchors = np.arange(c * S, c * S + S)
        pos = (anchors + B) % N
        used = np.zeros(N, dtype=bool)
        used[anchors] = True
        used[pos] = True
        loc = np.concatenate([anchors, pos, np.nonzero(~used)[0]])

        emb_loc = np.ascontiguousarray(emb[loc])
        embt = emb_loc.T
        ET = np.concatenate([embt[128 * k:128 * (k + 1), :] for k in range(4)],
                            axis=1).astype(f8)

        pmn = np.zeros((128, 192), dtype=np.float32)
        pmn[0:48, :] = -w2[anchors[:, None], loc[None, 0:192]]
        pmn[64:112, :] = -w2[anchors[:, None], loc[None, 192:384]]
        pmn[a48, a48] = 0.0          # k == i
        pmn[a48, S + a48] = 0.0      # k == p(i)

        in_maps.append({
            "et": ET,
            "pmw": pmn.astype(ml_dtypes.bfloat16),
            "selw": sels,
        })
    return in_maps


def reduce_outputs(results):
    parts = np.stack([np.asarray(r["out"][0], dtype=np.float64)
                      for r in results])
    total = parts.sum(axis=0)
    count = (total[1] + CELLS) / 2.0
    return np.asarray(
        np.float32(total[0]) / (np.float32(count) + np.float32(EPS)),
        dtype=np.float32)


def kernel(output1, output2, weight):
    in_maps = make_in_maps(output1, output2, weight)
    res = run_bass_kernel_spmd(_get_nc(), in_maps, core_ids=list(range(NCORES)))
    return reduce_outputs(res.results)
